# revision 19
# baseline (speedup 1.0000x reference)
"""Trainium2 Bass kernel for ragged box-attention (nn_Att_0_layer1).

Computation (see reference):
  logits[n,k] = w2 . relu(w1^T concat(v[n,k,:], q[n,:]) + b1) + b2
  padded      = ragged pad of logits rows into [B,S,T,K]
  out         = allennlp masked_softmax over K with box_mask

Strategy:
  - Data-parallel over B: core c owns groups [12c, 12c+12); its ragged rows
    are gathered host-side so the device only ever sees its own rows.
  - The FC contraction is split: the q-part (shared across the K=36 boxes of
    a row) is computed once per row and broadcast-added over boxes on-chip.
  - v is shipped transposed ([V_DIM, rows*K], bf16) so the contraction dim is
    the SBUF partition dim; weights are replicated and tiny.
  - b2 is dropped: a constant added to every logit cancels in the masked
    softmax (masked slots use exp(0) which is unaffected, and those slots are
    zeroed+renormalized away exactly).
  - Each group's "padding row" output equals mask/sum(mask); we get it from
    the same pipeline by appending one synthetic all-zero row per group.
  - Host scatters the compact per-row softmax results into [B,S,T,K].
"""

import numpy as np
import ml_dtypes
from contextlib import ExitStack

import concourse.bass as bass
import concourse.bacc as bacc
import concourse.tile as tile
from concourse import mybir
from concourse.bass_utils import run_bass_kernel_spmd

BF16_NP = ml_dtypes.bfloat16
F32 = mybir.dt.float32
BF16 = mybir.dt.bfloat16
# fp32r: 4-byte fp32 storage, PE runs it at bf16 speed for moving dim >= 256
# with near-fp32 numerics (HW-measured 1.4e-4 rel on a [128]x[128,504] mm)
IN_DT = mybir.dt.float32r
IN_NP = np.float32

N_CORES = 8
B, S, T, K = 96, 4, 16, 36
V_DIM, Q_DIM, H = 1024, 512, 512
GP = B // N_CORES            # groups per core
N_TILE = 14                  # n-rows per compute tile
COLS = N_TILE * K            # 504 matmul columns per tile (fills a PSUM bank)
DJ_V = V_DIM // 128          # 8 contraction chunks for the v part
DJ_Q = Q_DIM // 128          # 4 contraction chunks for the q part
HB = H // 128                # 4 hidden blocks
NEG = -1.0e4                 # log-mask for invalid boxes; exp(NEG+x) == 0.0

_CACHE: dict[int, "bacc.Bacc"] = {}


def _build(np_rows: int, rep: int = 1, loop_n: int = 0) -> "bacc.Bacc":
    """Build+compile the per-core program for np_rows padded n-rows.

    rep > 1 (static) or loop_n > 1 (hardware For_i loop) repeat the whole
    compute with identical results, for timing amplification; functional
    callers use rep=1, loop_n=0.
    """
    NP = np_rows
    NT = NP // N_TILE
    R = NP * K

    nc = bacc.Bacc("TRN2", target_bir_lowering=False, debug=False)

    vt = nc.dram_tensor("vt", [V_DIM, R], IN_DT, kind="ExternalInput")
    qt = nc.dram_tensor("qt", [Q_DIM, NP], IN_DT, kind="ExternalInput")
    w1v = nc.dram_tensor("w1v", [V_DIM, H], IN_DT, kind="ExternalInput")
    w1q = nc.dram_tensor("w1q", [Q_DIM, H], IN_DT, kind="ExternalInput")
    b1r = nc.dram_tensor("b1r", [128, HB], F32, kind="ExternalInput")
    w2r = nc.dram_tensor("w2r", [128, HB], IN_DT, kind="ExternalInput")
    lmask = nc.dram_tensor("lmask", [NP, K], F32, kind="ExternalInput")
    lg_dram = nc.dram_tensor("lg_dram", [R], F32, kind="ExternalOutput")
    wout = nc.dram_tensor("wout", [NP, K], F32, kind="ExternalOutput")

    with tile.TileContext(nc) as tc, ExitStack() as ctx:
        wpool = ctx.enter_context(tc.tile_pool(name="weights", bufs=1))
        vin = ctx.enter_context(tc.tile_pool(name="vin", bufs=4))
        psum = ctx.enter_context(tc.tile_pool(name="psum", bufs=6, space="PSUM"))
        psl = ctx.enter_context(tc.tile_pool(name="psl", bufs=2, space="PSUM"))
        jp = ctx.enter_context(tc.tile_pool(name="joint", bufs=4))
        lgp = ctx.enter_context(tc.tile_pool(name="lg", bufs=4))
        smx = ctx.enter_context(tc.tile_pool(name="smx", bufs=3))

        # --- resident weights / per-row q activations ---
        w1v_sb = wpool.tile([128, DJ_V, H], IN_DT)
        nc.sync.dma_start(out=w1v_sb, in_=w1v.ap().rearrange("(j p) h -> p j h", p=128))
        w1q_sb = wpool.tile([128, DJ_Q, H], IN_DT)
        nc.sync.dma_start(out=w1q_sb, in_=w1q.ap().rearrange("(j p) h -> p j h", p=128))
        b1_sb = wpool.tile([128, HB], F32)
        nc.sync.dma_start(out=b1_sb, in_=b1r.ap())
        w2_sb = wpool.tile([128, HB], IN_DT)
        nc.sync.dma_start(out=w2_sb, in_=w2r.ap())
        qt_sb = wpool.tile([128, DJ_Q, NP], IN_DT)
        nc.sync.dma_start(out=qt_sb, in_=qt.ap().rearrange("(j p) n -> p j n", p=128))

        # qh[h, n] = w1q^T q^T, accumulated over the 4 q-contraction chunks
        qh_sb = wpool.tile([128, HB, NP], F32)
        QC = 420
        if loop_n:
            with tc.For_i(0, loop_n, 1):
                _main_pass(nc, tc, ctx, NP, qh_sb, w1v_sb, w1q_sb, b1_sb,
                           w2_sb, qt_sb, vt, lmask, lg_dram, wout,
                           psum, psl, vin, jp, lgp, smx, QC)
        else:
            for _rep in range(rep):
                _main_pass(nc, tc, ctx, NP, qh_sb, w1v_sb, w1q_sb, b1_sb,
                           w2_sb, qt_sb, vt, lmask, lg_dram, wout,
                           psum, psl, vin, jp, lgp, smx, QC)

    nc.compile()
    return nc


def _main_pass(nc, tc, ctx, NP, qh_sb, w1v_sb, w1q_sb, b1_sb, w2_sb,
               qt_sb, vt, lmask, lg_dram, wout,
               psum, psl, vin, jp, lgp, smx, QC):
    NT = NP // N_TILE
    R = NP * K
    lg_rows = lg_dram.ap().rearrange("(n k) -> n k", k=K)
    vt_r = vt.ap().rearrange("(j p) c -> p j c", p=128)
    jtiles = [None] * NT

    # softmax chunk a (rows [128a, 128a+128)) becomes runnable once the last
    # logits tile covering it has been stored to lg_dram
    n_chunks = -(-NP // 128)
    chunk_ready_tile = [
        min(NT - 1, -(-(128 * (a + 1)) // N_TILE) - 1) for a in range(n_chunks)
    ]

    def emit_softmax_chunk(a):
        st = a * 128
        rows = min(128, NP - st)
        lg = smx.tile([128, K], F32, name="lg")[:rows]
        nc.sync.dma_start(out=lg, in_=lg_rows[st:st + rows, :])
        lm = smx.tile([128, K], F32, name="lm")[:rows]
        nc.sync.dma_start(out=lm, in_=lmask.ap()[st:st + rows, :])
        el = smx.tile([128, K], F32, name="el")[:rows]
        sums = smx.tile([128, 1], F32, name="sums")[:rows]
        nc.vector.tensor_add(lg, lg, lm)
        nc.scalar.activation(el, lg, mybir.ActivationFunctionType.Exp,
                             accum_out=sums)
        rec = smx.tile([128, 1], F32, name="rec")[:rows]
        nc.vector.reciprocal(rec, sums)
        wt = smx.tile([128, K], F32, name="wt")[:rows]
        nc.vector.tensor_scalar_mul(wt, el, rec)
        nc.sync.dma_start(out=wout.ap()[st:st + rows, :], in_=wt)

    def emit_logits(t):
        pl = psl.tile([1, COLS], F32)
        for hb in range(HB):
            nc.tensor.matmul(
                pl,
                lhsT=w2_sb[:, hb:hb + 1],
                rhs=jtiles[t][:, hb, :],
                start=(hb == 0),
                stop=(hb == HB - 1),
            )
        lst = lgp.tile([1, COLS], F32, name="lst")
        nc.vector.tensor_copy(lst, pl)
        nc.sync.dma_start(out=lg_dram.ap()[t * COLS:(t + 1) * COLS], in_=lst)
        jtiles[t] = None
        for a in range(n_chunks):
            if chunk_ready_tile[a] == t:
                emit_softmax_chunk(a)

    def emit_qphase():
        for hb in range(HB):
            for c0 in range(0, NP, QC):
                cw = min(QC, NP - c0)
                ps = psum.tile([128, COLS], F32, name="ps")[:, :cw]
                for dj in range(DJ_Q):
                    nc.tensor.matmul(
                        ps,
                        lhsT=w1q_sb[:, dj, hb * 128:(hb + 1) * 128],
                        rhs=qt_sb[:, dj, c0:c0 + cw],
                        start=(dj == 0),
                        stop=(dj == DJ_Q - 1),
                    )
                nc.scalar.copy(qh_sb[:, hb, c0:c0 + cw], ps)

    for t in range(NT):
        c0 = t * COLS
        vtile = vin.tile([128, DJ_V, COLS], IN_DT)
        nc.sync.dma_start(out=vtile, in_=vt_r[:, :, c0:c0 + COLS])
        if t == 0:
            # q-part runs on the PE while the first v tiles stream in
            emit_qphase()
        jtile = jp.tile([128, HB, COLS], IN_DT)
        for hb in range(HB):
            ps = psum.tile([128, COLS], F32, name="ps")
            for dj in range(DJ_V):
                nc.tensor.matmul(
                    ps,
                    lhsT=w1v_sb[:, dj, hb * 128:(hb + 1) * 128],
                    rhs=vtile[:, dj, :],
                    start=(dj == 0),
                    stop=(dj == DJ_V - 1),
                )
            ps3 = ps.rearrange("p (n k) -> p n k", k=K)
            qb = qh_sb[:, hb, t * N_TILE:(t + 1) * N_TILE].to_broadcast(
                [128, N_TILE, K])
            nc.vector.tensor_add(ps3, ps3, qb)
            nc.scalar.activation(
                jtile[:, hb, :], ps, mybir.ActivationFunctionType.Relu,
                bias=b1_sb[:, hb:hb + 1], scale=1.0)
        jtiles[t] = jtile
        # logits for tile t-1 land after tile t's v-matmuls so the PE
        # never waits on the DVE/ACT epilogue of its own tile
        if t > 0:
            emit_logits(t - 1)
    emit_logits(NT - 1)


def prepare(v, q, box_mask, tags_attention, w1, b1, w2):
    """Host-side shard/layout prep. Returns (NP, in_maps, scatter_idx)."""
    N = v.shape[0]
    lengths = tags_attention.sum(-1).reshape(-1).astype(np.int64)  # [B*S]
    starts = np.cumsum(lengths) - lengths

    # per-core ragged row index lists (clipped exactly like the reference)
    rows_per_core = []
    for c in range(N_CORES):
        idx = []
        for slot in range(c * GP * S, (c + 1) * GP * S):
            ln = int(lengths[slot])
            if ln > 0:
                r = starts[slot] + np.arange(ln)
                idx.append(np.clip(r, 0, N - 1))
        rows_per_core.append(
            np.concatenate(idx) if idx else np.zeros(0, np.int64))

    n_aug = [len(r) + GP for r in rows_per_core]  # + synthetic pad-rows
    NP = -(-max(n_aug) // N_TILE) * N_TILE

    w1v_h = np.ascontiguousarray(w1[:V_DIM], dtype=IN_NP)
    w1q_h = np.ascontiguousarray(w1[V_DIM:], dtype=IN_NP)
    b1r_h = np.ascontiguousarray(b1.reshape(HB, 128).T)
    w2r_h = np.ascontiguousarray(w2.reshape(HB, 128).T, dtype=IN_NP)
    lmask_full = np.where(box_mask > 0, np.float32(0.0), np.float32(NEG))

    def prep_core(c):
        rows = rows_per_core[c]
        n_c = len(rows)
        # contiguous row ranges can be sliced instead of gathered
        if n_c and np.all(np.diff(rows) == 1):
            v_rows = v[rows[0]:rows[0] + n_c]
            q_rows = q[rows[0]:rows[0] + n_c]
        else:
            v_rows = v[rows]
            q_rows = q[rows]
        vt_h = np.zeros((V_DIM, NP * K), dtype=IN_NP)
        vt_h[:, :n_c * K] = np.asarray(v_rows, dtype=IN_NP).reshape(n_c * K, V_DIM).T
        qt_h = np.zeros((Q_DIM, NP), dtype=IN_NP)
        qt_h[:, :n_c] = np.asarray(q_rows, dtype=IN_NP).T

        lmask_h = np.zeros((NP, K), dtype=np.float32)
        gidx = np.repeat(np.arange(c * GP, (c + 1) * GP), S)  # group per slot
        slot_lens = lengths[c * GP * S:(c + 1) * GP * S]
        row_groups = np.repeat(gidx, slot_lens)               # group per row
        lmask_h[:n_c] = lmask_full[row_groups]
        lmask_h[n_c:n_c + GP] = lmask_full[c * GP:(c + 1) * GP]

        in_map = {
            "vt": vt_h, "qt": qt_h, "w1v": w1v_h, "w1q": w1q_h,
            "b1r": b1r_h, "w2r": w2r_h, "lmask": lmask_h,
        }

        # map (g_local, s, t) -> compact row position
        pos = np.full((GP * S, T), 0, dtype=np.int64)
        off = 0
        for ls in range(GP * S):
            ln = int(slot_lens[ls])
            g_local = ls // S
            pos[ls, :] = n_c + g_local                  # synthetic pad-row
            pos[ls, :ln] = off + np.arange(ln)
            off += ln
        return in_map, pos.reshape(GP, S, T)

    from concurrent.futures import ThreadPoolExecutor
    with ThreadPoolExecutor(max_workers=N_CORES) as ex:
        results = list(ex.map(prep_core, range(N_CORES)))
    in_maps = [r[0] for r in results]
    scatter_idx = [r[1] for r in results]

    return NP, in_maps, scatter_idx


def kernel(v, q, box_mask, tags_attention, w1, b1, w2, b2):
    v = np.asarray(v)
    q = np.asarray(q)
    box_mask = np.asarray(box_mask, dtype=np.float32)
    tags_attention = np.asarray(tags_attention)
    w1 = np.asarray(w1, dtype=np.float32)
    b1 = np.asarray(b1, dtype=np.float32)
    w2 = np.asarray(w2, dtype=np.float32)

    NP, in_maps, scatter_idx = prepare(v, q, box_mask, tags_attention, w1, b1, w2)

    nc = _CACHE.get(NP)
    if nc is None:
        nc = _CACHE[NP] = _build(NP)

    res = run_bass_kernel_spmd(nc, in_maps, core_ids=list(range(N_CORES)))

    out = np.empty((B, S, T, K), dtype=np.float32)
    for c in range(N_CORES):
        w_c = res.results[c]["wout"]                     # [NP, K]
        out[c * GP:(c + 1) * GP] = w_c[scatter_idx[c]]
    return out


# revision 23
# speedup vs baseline: 1.0806x; 1.0806x over previous
"""Trainium2 Bass kernel for ragged box-attention (nn_Att_0_layer1).

Computation (see reference):
  logits[n,k] = w2 . relu(w1^T concat(v[n,k,:], q[n,:]) + b1) + b2
  padded      = ragged pad of logits rows into [B,S,T,K]
  out         = allennlp masked_softmax over K with box_mask

Strategy:
  - Data-parallel over B: core c owns groups [12c, 12c+12); its ragged rows
    are gathered host-side so the device only ever sees its own rows.
  - The FC contraction is split: the q-part (shared across the K=36 boxes of
    a row) is computed once per row and broadcast-added over boxes on-chip.
  - v is shipped transposed ([V_DIM, rows*K], bf16) so the contraction dim is
    the SBUF partition dim; weights are replicated and tiny.
  - b2 is dropped: a constant added to every logit cancels in the masked
    softmax (masked slots use exp(0) which is unaffected, and those slots are
    zeroed+renormalized away exactly).
  - Each group's "padding row" output equals mask/sum(mask); we get it from
    the same pipeline by appending one synthetic all-zero row per group.
  - Host scatters the compact per-row softmax results into [B,S,T,K].
"""

import numpy as np
import ml_dtypes
from contextlib import ExitStack

import concourse.bass as bass
import concourse.bacc as bacc
import concourse.tile as tile
from concourse import mybir
from concourse.bass_utils import run_bass_kernel_spmd

BF16_NP = ml_dtypes.bfloat16
F32 = mybir.dt.float32
BF16 = mybir.dt.bfloat16
# fp32r: 4-byte fp32 storage, PE runs it at bf16 speed for moving dim >= 256
# with near-fp32 numerics (HW-measured 1.4e-4 rel on a [128]x[128,504] mm)
IN_DT = mybir.dt.float32r
IN_NP = np.float32

N_CORES = 8
B, S, T, K = 96, 4, 16, 36
V_DIM, Q_DIM, H = 1024, 512, 512
GP = B // N_CORES            # groups per core
N_TILE = 14                  # n-rows per compute tile
COLS = N_TILE * K            # 504 matmul columns per tile (fills a PSUM bank)
DJ_V = V_DIM // 128          # 8 contraction chunks for the v part
DJ_Q = Q_DIM // 128          # 4 contraction chunks for the q part
HB = H // 128                # 4 hidden blocks
NEG = -1.0e4                 # log-mask for invalid boxes; exp(NEG+x) == 0.0

_CACHE: dict[int, "bacc.Bacc"] = {}


def _build(np_rows: int, rep: int = 1, loop_n: int = 0) -> "bacc.Bacc":
    """Build+compile the per-core program for np_rows padded n-rows.

    rep > 1 (static) or loop_n > 1 (hardware For_i loop) repeat the whole
    compute with identical results, for timing amplification; functional
    callers use rep=1, loop_n=0.
    """
    NP = np_rows
    NT = NP // N_TILE
    R = NP * K

    nc = bacc.Bacc("TRN2", target_bir_lowering=False, debug=False)

    vt = nc.dram_tensor("vt", [V_DIM, R], IN_DT, kind="ExternalInput")
    qt = nc.dram_tensor("qt", [Q_DIM, NP], IN_DT, kind="ExternalInput")
    w1v = nc.dram_tensor("w1v", [V_DIM, H], IN_DT, kind="ExternalInput")
    w1q = nc.dram_tensor("w1q", [Q_DIM, H], IN_DT, kind="ExternalInput")
    b1r = nc.dram_tensor("b1r", [128, HB], F32, kind="ExternalInput")
    w2r = nc.dram_tensor("w2r", [128, HB], IN_DT, kind="ExternalInput")
    lmask = nc.dram_tensor("lmask", [NP, K], F32, kind="ExternalInput")
    onesr = nc.dram_tensor("onesr", [128, 1], IN_DT, kind="ExternalInput")
    lg_dram = nc.dram_tensor("lg_dram", [R], F32, kind="ExternalOutput")
    wout = nc.dram_tensor("wout", [NP, K], F32, kind="ExternalOutput")

    with tile.TileContext(nc) as tc, ExitStack() as ctx:
        wpool = ctx.enter_context(tc.tile_pool(name="weights", bufs=1))
        vin = ctx.enter_context(tc.tile_pool(name="vin", bufs=4))
        psum = ctx.enter_context(tc.tile_pool(name="psum", bufs=6, space="PSUM"))
        psl = ctx.enter_context(tc.tile_pool(name="psl", bufs=2, space="PSUM"))
        jp = ctx.enter_context(tc.tile_pool(name="joint", bufs=4))
        lgp = ctx.enter_context(tc.tile_pool(name="lg", bufs=4))
        smx = ctx.enter_context(tc.tile_pool(name="smx", bufs=3))
        fold = ctx.enter_context(tc.tile_pool(name="fold", bufs=3))

        # --- resident weights / per-row q activations ---
        # qt + w1q land first so the q-phase can start while w1v/v stream in
        qt_sb = wpool.tile([128, DJ_Q, NP], IN_DT)
        nc.sync.dma_start(out=qt_sb, in_=qt.ap().rearrange("(j p) n -> p j n", p=128))
        w1q_sb = wpool.tile([128, DJ_Q, H], IN_DT)
        nc.sync.dma_start(out=w1q_sb, in_=w1q.ap().rearrange("(j p) h -> p j h", p=128))
        b1_sb = wpool.tile([128, HB], F32)
        nc.sync.dma_start(out=b1_sb, in_=b1r.ap())
        w2_sb = wpool.tile([128, HB], IN_DT)
        nc.sync.dma_start(out=w2_sb, in_=w2r.ap())
        w1v_sb = wpool.tile([128, DJ_V, H], IN_DT)
        nc.sync.dma_start(out=w1v_sb, in_=w1v.ap().rearrange("(j p) h -> p j h", p=128))
        ones_sb = wpool.tile([128, 1], IN_DT)
        nc.sync.dma_start(out=ones_sb, in_=onesr.ap())
        w2f_sb = wpool.tile([128, HB], F32)
        nc.sync.dma_start(out=w2f_sb, in_=w2r.ap().bitcast(F32))

        # qh[h, n] = w1q^T q^T, accumulated over the 4 q-contraction chunks
        qh_sb = wpool.tile([128, HB, NP], F32)
        QC = 420
        if loop_n:
            with tc.For_i(0, loop_n, 1):
                _main_pass(nc, tc, ctx, NP, qh_sb, w1v_sb, w1q_sb, b1_sb,
                           w2f_sb, ones_sb, qt_sb, vt, lmask, lg_dram, wout,
                           psum, psl, vin, jp, lgp, smx, fold, QC)
        else:
            for _rep in range(rep):
                _main_pass(nc, tc, ctx, NP, qh_sb, w1v_sb, w1q_sb, b1_sb,
                           w2f_sb, ones_sb, qt_sb, vt, lmask, lg_dram, wout,
                           psum, psl, vin, jp, lgp, smx, fold, QC)

    nc.compile()
    return nc


def _main_pass(nc, tc, ctx, NP, qh_sb, w1v_sb, w1q_sb, b1_sb, w2f_sb,
               ones_sb, qt_sb, vt, lmask, lg_dram, wout,
               psum, psl, vin, jp, lgp, smx, fold, QC):
    NT = NP // N_TILE
    R = NP * K
    lg_rows = lg_dram.ap().rearrange("(n k) -> n k", k=K)
    vt_r = vt.ap().rearrange("(j p) c -> p j c", p=128)
    jtiles = [None] * NT

    # softmax chunk a (rows [128a, 128a+128)) becomes runnable once the last
    # logits tile covering it has been stored to lg_dram
    n_chunks = -(-NP // 128)
    chunk_ready_tile = [
        min(NT - 1, -(-(128 * (a + 1)) // N_TILE) - 1) for a in range(n_chunks)
    ]

    def emit_softmax_chunk(a):
        st = a * 128
        rows = min(128, NP - st)
        lg = smx.tile([128, K], F32, name="lg")[:rows]
        nc.sync.dma_start(out=lg, in_=lg_rows[st:st + rows, :])
        lm = smx.tile([128, K], F32, name="lm")[:rows]
        nc.sync.dma_start(out=lm, in_=lmask.ap()[st:st + rows, :])
        el = smx.tile([128, K], F32, name="el")[:rows]
        sums = smx.tile([128, 1], F32, name="sums")[:rows]
        nc.vector.tensor_add(lg, lg, lm)
        nc.scalar.activation(el, lg, mybir.ActivationFunctionType.Exp,
                             accum_out=sums)
        rec = smx.tile([128, 1], F32, name="rec")[:rows]
        nc.vector.reciprocal(rec, sums)
        wt = smx.tile([128, K], F32, name="wt")[:rows]
        nc.vector.tensor_scalar_mul(wt, el, rec)
        nc.sync.dma_start(out=wout.ap()[st:st + rows, :], in_=wt)

    def emit_logits(t):
        # fold the 4 h-blocks with their w2 weights on the DVE (per-partition
        # scalars), then a single ones-vector matmul does the partition sum;
        # acc stays native float32r so the BIR verifier sees rounded input
        acc = fold.tile([128, COLS], IN_DT, name="acc")
        jt = jtiles[t]
        nc.vector.tensor_scalar_mul(acc, jt[:, 0, :], w2f_sb[:, 0:1])
        for hb in range(1, HB):
            nc.vector.scalar_tensor_tensor(
                out=acc, in0=jt[:, hb, :],
                scalar=w2f_sb[:, hb:hb + 1], in1=acc,
                op0=mybir.AluOpType.mult, op1=mybir.AluOpType.add)
        pl = psl.tile([1, COLS], F32)
        nc.tensor.matmul(pl, lhsT=ones_sb, rhs=acc, start=True, stop=True)
        lst = lgp.tile([1, COLS], F32, name="lst")
        nc.vector.tensor_copy(lst, pl)
        nc.sync.dma_start(out=lg_dram.ap()[t * COLS:(t + 1) * COLS], in_=lst)
        jtiles[t] = None
        for a in range(n_chunks):
            if chunk_ready_tile[a] == t:
                emit_softmax_chunk(a)

    def emit_qphase():
        for hb in range(HB):
            for c0 in range(0, NP, QC):
                cw = min(QC, NP - c0)
                ps = psum.tile([128, COLS], F32, name="ps")[:, :cw]
                for dj in range(DJ_Q):
                    nc.tensor.matmul(
                        ps,
                        lhsT=w1q_sb[:, dj, hb * 128:(hb + 1) * 128],
                        rhs=qt_sb[:, dj, c0:c0 + cw],
                        start=(dj == 0),
                        stop=(dj == DJ_Q - 1),
                    )
                nc.scalar.copy(qh_sb[:, hb, c0:c0 + cw], ps)

    for t in range(NT):
        c0 = t * COLS
        vtile = vin.tile([128, DJ_V, COLS], IN_DT)
        nc.sync.dma_start(out=vtile, in_=vt_r[:, :, c0:c0 + COLS])
        if t == 0:
            # q-part runs on the PE while the first v tiles stream in
            emit_qphase()
        jtile = jp.tile([128, HB, COLS], IN_DT)
        for hb in range(HB):
            ps = psum.tile([128, COLS], F32, name="ps")
            for dj in range(DJ_V):
                nc.tensor.matmul(
                    ps,
                    lhsT=w1v_sb[:, dj, hb * 128:(hb + 1) * 128],
                    rhs=vtile[:, dj, :],
                    start=(dj == 0),
                    stop=(dj == DJ_V - 1),
                )
            ps3 = ps.rearrange("p (n k) -> p n k", k=K)
            qb = qh_sb[:, hb, t * N_TILE:(t + 1) * N_TILE].to_broadcast(
                [128, N_TILE, K])
            nc.vector.tensor_add(ps3, ps3, qb)
            nc.scalar.activation(
                jtile[:, hb, :], ps, mybir.ActivationFunctionType.Relu,
                bias=b1_sb[:, hb:hb + 1], scale=1.0)
        jtiles[t] = jtile
        # logits for tile t-1 land after tile t's v-matmuls so the PE
        # never waits on the DVE/ACT epilogue of its own tile
        if t > 0:
            emit_logits(t - 1)
    emit_logits(NT - 1)


def prepare(v, q, box_mask, tags_attention, w1, b1, w2):
    """Host-side shard/layout prep. Returns (NP, in_maps, scatter_idx)."""
    N = v.shape[0]
    lengths = tags_attention.sum(-1).reshape(-1).astype(np.int64)  # [B*S]
    starts = np.cumsum(lengths) - lengths

    # per-core ragged row index lists (clipped exactly like the reference)
    rows_per_core = []
    for c in range(N_CORES):
        idx = []
        for slot in range(c * GP * S, (c + 1) * GP * S):
            ln = int(lengths[slot])
            if ln > 0:
                r = starts[slot] + np.arange(ln)
                idx.append(np.clip(r, 0, N - 1))
        rows_per_core.append(
            np.concatenate(idx) if idx else np.zeros(0, np.int64))

    n_aug = [len(r) + GP for r in rows_per_core]  # + synthetic pad-rows
    NP = -(-max(n_aug) // N_TILE) * N_TILE

    w1v_h = np.ascontiguousarray(w1[:V_DIM], dtype=IN_NP)
    w1q_h = np.ascontiguousarray(w1[V_DIM:], dtype=IN_NP)
    b1r_h = np.ascontiguousarray(b1.reshape(HB, 128).T)
    w2r_h = np.ascontiguousarray(w2.reshape(HB, 128).T, dtype=IN_NP)
    lmask_full = np.where(box_mask > 0, np.float32(0.0), np.float32(NEG))

    def prep_core(c):
        rows = rows_per_core[c]
        n_c = len(rows)
        # contiguous row ranges can be sliced instead of gathered
        if n_c and np.all(np.diff(rows) == 1):
            v_rows = v[rows[0]:rows[0] + n_c]
            q_rows = q[rows[0]:rows[0] + n_c]
        else:
            v_rows = v[rows]
            q_rows = q[rows]
        vt_h = np.zeros((V_DIM, NP * K), dtype=IN_NP)
        vt_h[:, :n_c * K] = np.asarray(v_rows, dtype=IN_NP).reshape(n_c * K, V_DIM).T
        qt_h = np.zeros((Q_DIM, NP), dtype=IN_NP)
        qt_h[:, :n_c] = np.asarray(q_rows, dtype=IN_NP).T

        lmask_h = np.zeros((NP, K), dtype=np.float32)
        gidx = np.repeat(np.arange(c * GP, (c + 1) * GP), S)  # group per slot
        slot_lens = lengths[c * GP * S:(c + 1) * GP * S]
        row_groups = np.repeat(gidx, slot_lens)               # group per row
        lmask_h[:n_c] = lmask_full[row_groups]
        lmask_h[n_c:n_c + GP] = lmask_full[c * GP:(c + 1) * GP]

        in_map = {
            "vt": vt_h, "qt": qt_h, "w1v": w1v_h, "w1q": w1q_h,
            "b1r": b1r_h, "w2r": w2r_h, "lmask": lmask_h,
            "onesr": np.ones((128, 1), np.float32),
        }

        # map (g_local, s, t) -> compact row position
        pos = np.full((GP * S, T), 0, dtype=np.int64)
        off = 0
        for ls in range(GP * S):
            ln = int(slot_lens[ls])
            g_local = ls // S
            pos[ls, :] = n_c + g_local                  # synthetic pad-row
            pos[ls, :ln] = off + np.arange(ln)
            off += ln
        return in_map, pos.reshape(GP, S, T)

    from concurrent.futures import ThreadPoolExecutor
    with ThreadPoolExecutor(max_workers=N_CORES) as ex:
        results = list(ex.map(prep_core, range(N_CORES)))
    in_maps = [r[0] for r in results]
    scatter_idx = [r[1] for r in results]

    return NP, in_maps, scatter_idx


def kernel(v, q, box_mask, tags_attention, w1, b1, w2, b2):
    v = np.asarray(v)
    q = np.asarray(q)
    box_mask = np.asarray(box_mask, dtype=np.float32)
    tags_attention = np.asarray(tags_attention)
    w1 = np.asarray(w1, dtype=np.float32)
    b1 = np.asarray(b1, dtype=np.float32)
    w2 = np.asarray(w2, dtype=np.float32)

    NP, in_maps, scatter_idx = prepare(v, q, box_mask, tags_attention, w1, b1, w2)

    nc = _CACHE.get(NP)
    if nc is None:
        nc = _CACHE[NP] = _build(NP)

    res = run_bass_kernel_spmd(nc, in_maps, core_ids=list(range(N_CORES)))

    out = np.empty((B, S, T, K), dtype=np.float32)
    for c in range(N_CORES):
        w_c = res.results[c]["wout"]                     # [NP, K]
        out[c * GP:(c + 1) * GP] = w_c[scatter_idx[c]]
    return out
